# revision 1
# baseline (speedup 1.0000x reference)
"""Trainium2 Bass kernel for nn_ActorMultiHead (moe_routing).

Strategy
--------
The reference runs every role head on every token (dense form of a masked
dispatch) and then selects the row matching the token's role; tokens whose
role >= NUM_ROLES contribute exactly 0.  We implement the sparse dispatch:

  * Host: flatten [B, A] tokens, drop role>=2 tokens (their output is 0),
    sort the rest by role, pack them into 512-token single-role tiles, and
    assign whole tiles to cores so every core serves exactly one role.
    The one-hot input contribution then folds into the layer-0 bias and the
    per-role heads become per-core weight *data* — the SPMD graph is
    role-agnostic.
  * Device (per core, 6 tiles x 512 tokens), mixed precision:
        layer 0 (obs->H)      : f32r matmul, contraction 128
                                (bf16 L0 miscomputes on TRN2 in this
                                graph; f32r is exact and nearly as fast)
        layers 1,2 (H->H)     : fp8e4m3 DoubleRow matmuls (contraction 256
                                per instruction -> 4 matmuls per out-block)
        head 1 (H->H2)        : fp8 DoubleRow
        head 2 (H2->act)      : fp8 DoubleRow (M padded 8->16 so the weight
                                plane stride stays 16B-aligned)
        log-prob reduce       : f32r matmul with lhsT=-0.5*inv_std^2
    fp8 scaling: every fp8-layer PSUM uniformly holds P*u (P=16).  ReLU
    blocks are split between ScalarE (free scale+bias -> stores alpha*h,
    alpha=4) and VectorE (single tensor_scalar add+max -> stores P*h or
    1*h for layer 0), and the per-block scale is folded into the NEXT
    layer's fp8 weight quantization (c_k = P / s_block(k)).
    Tiles are processed in pairs, interleaved per out-block, so each
    engine's work on tile A overlaps the other engines' work on tile B and
    the PE never waits on an activation chain; per-tile tails (tanh, diff,
    square, reduce) are deferred into the next pair's matmul body.
  * Host: scatter per-tile outputs back to original token positions.

For timing, a variant graph wraps the whole per-core compute in a
constant-bound For_i loop (set `kernel.nrep = R`), so a test harness can
measure HW time as (wall(nrep=R) - wall(nrep=1)) / (R-1).
"""

import math

import numpy as np
import ml_dtypes

# -- problem constants (from the problem statement, hardcoded) ---------------
B, A = 2048, 16
OBS_DIM, HIDDEN, ACTION_DIM = 128, 1024, 8
NUM_ROLES = 2
AGENT_ID_DIM = NUM_ROLES
H2 = HIDDEN // 2
LOG_2PI = math.log(2.0 * math.pi)

N_CORES = 8
NT = 512          # tokens per tile (matmul moving free dim)
CT = 6            # tiles per core (fixed compile-time shape)
C = NT * CT       # tokens per core per batch
KH = HIDDEN // 128    # 8 feature blocks of hidden
KZ = H2 // 128        # 4 feature blocks of the head hidden
KKH = KH // 2         # 4 DoubleRow k-pair tiles over hidden
KKZ = KZ // 2         # 2 DoubleRow k-pair tiles over head hidden
MPAD = 16             # head-2 output padded to 16 for DoubleRow alignment

P_SC = 16.0       # uniform fp8-layer PSUM scale
ALPHA = 4.0       # ScalarE-block stored-activation scale

# per-block engine maps: True -> ScalarE (activation), False -> VectorE
MAP_L0 = (True, True, True, True, True, False, False, False)
MAP_L1 = (True, True, True, True, True, False, False, False)
MAP_L2 = (True, True, True, True, True, False, False, False)
MAP_H1 = (True, True, False, False)

F8 = ml_dtypes.float8_e4m3
BF16 = ml_dtypes.bfloat16

PAIRED = True  # process tiles in interleaved pairs (set False to bisect)
DEFER_TAILS = True  # defer per-tile tails into the next pair's matmul body

_GRAPHS = {}  # repeats -> compiled graph, built once per process


def _build_graph(repeats=1):
    import concourse.bass as bass
    import concourse.tile as tile
    from concourse import bacc, mybir

    f32 = mybir.dt.float32
    f32r = mybir.dt.float32r
    bf16 = mybir.dt.bfloat16
    fp8 = mybir.dt.float8e4
    Act = mybir.ActivationFunctionType
    Alu = mybir.AluOpType
    DR = mybir.MatmulPerfMode.DoubleRow

    nc = bacc.Bacc(None, target_bir_lowering=False)

    xT = nc.declare_dram_parameter("xT", [CT, 128, NT], f32r, isOutput=False)
    aT = nc.declare_dram_parameter("aT", [ACTION_DIM, C], f32, isOutput=False)
    w0 = nc.declare_dram_parameter("w0", [128, HIDDEN], f32r, isOutput=False)
    bias0 = nc.declare_dram_parameter("bias0", [128, KH], f32, isOutput=False)
    w1 = nc.declare_dram_parameter("w1", [128, KKH, 2, HIDDEN], fp8, isOutput=False)
    bias1 = nc.declare_dram_parameter("bias1", [128, KH], f32, isOutput=False)
    w2 = nc.declare_dram_parameter("w2", [128, KKH, 2, HIDDEN], fp8, isOutput=False)
    bias2 = nc.declare_dram_parameter("bias2", [128, KH], f32, isOutput=False)
    hw1 = nc.declare_dram_parameter("hw1", [128, KKH, 2, H2], fp8, isOutput=False)
    hbias1 = nc.declare_dram_parameter("hbias1", [128, KZ], f32, isOutput=False)
    hw2 = nc.declare_dram_parameter("hw2", [128, KKZ, 2, MPAD], fp8, isOutput=False)
    hb2 = nc.declare_dram_parameter("hb2", [ACTION_DIM, 1], f32, isOutput=False)
    wred = nc.declare_dram_parameter("wred", [ACTION_DIM, 1], f32r, isOutput=False)
    cc = nc.declare_dram_parameter("cc", [1, 1], f32, isOutput=False)
    out = nc.declare_dram_parameter("out", [CT, NT], f32, isOutput=True)

    with tile.TileContext(nc) as tc:
        with (
            tc.tile_pool(name="consts", bufs=1) as consts,
            tc.tile_pool(name="acts", bufs=2) as acts,
            tc.tile_pool(name="small", bufs=3) as small,
            tc.tile_pool(name="psum", bufs=4, space="PSUM") as psum,
            tc.tile_pool(name="psmall", bufs=2, space="PSUM") as psmall,
        ):
            # resident weights / biases
            w0_sb = consts.tile([128, HIDDEN], f32r)
            nc.sync.dma_start(w0_sb[:], w0[:])
            w1_sb = consts.tile([128, KKH, 2, HIDDEN], fp8)
            nc.sync.dma_start(w1_sb[:], w1[:])
            w2_sb = consts.tile([128, KKH, 2, HIDDEN], fp8)
            nc.sync.dma_start(w2_sb[:], w2[:])
            hw1_sb = consts.tile([128, KKH, 2, H2], fp8)
            nc.sync.dma_start(hw1_sb[:], hw1[:])
            hw2_sb = consts.tile([128, KKZ, 2, MPAD], fp8)
            nc.sync.dma_start(hw2_sb[:], hw2[:])
            b0_sb = consts.tile([128, KH], f32)
            nc.sync.dma_start(b0_sb[:], bias0[:])
            b1_sb = consts.tile([128, KH], f32)
            nc.sync.dma_start(b1_sb[:], bias1[:])
            b2_sb = consts.tile([128, KH], f32)
            nc.sync.dma_start(b2_sb[:], bias2[:])
            hb1_sb = consts.tile([128, KZ], f32)
            nc.sync.dma_start(hb1_sb[:], hbias1[:])
            hb2_sb = consts.tile([ACTION_DIM, 1], f32)
            nc.sync.dma_start(hb2_sb[:], hb2[:])
            wred_sb = consts.tile([ACTION_DIM, 1], f32r)
            nc.sync.dma_start(wred_sb[:], wred[:])
            cc_sb = consts.tile([1, 1], f32)
            nc.sync.dma_start(cc_sb[:], cc[:])

            xt_sb = []
            for t in range(CT):
                xt = consts.tile([128, NT], f32r, name=f"xt{t}", uniquify=True)
                nc.sync.dma_start(xt[:], xT[t])
                xt_sb.append(xt)
            aT_sb = consts.tile([ACTION_DIM, C], f32)
            nc.sync.dma_start(aT_sb[:], aT[:])

            from contextlib import nullcontext

            def emit_relu(dest, m, ps, bias_sb, scaled, emap):
                """dest[:, m//2, m%2, :] = stored activation for block m."""
                dst = dest[:, m // 2, m % 2, :]
                bcol = bias_sb[:, m : m + 1]
                if emap[m]:
                    nc.scalar.activation(
                        dst, ps[:], Act.Relu, bias=bcol,
                        scale=(ALPHA / P_SC) if scaled else ALPHA,
                    )
                else:
                    nc.vector.tensor_scalar(
                        dst, ps[:], bcol, 0.0, Alu.add, Alu.max
                    )

            def emit_tail(t, pm):
                ts = bass.ts(t, NT)
                mean = small.tile([ACTION_DIM, NT], f32, tag="mean")
                nc.scalar.activation(
                    mean[:], pm[0:ACTION_DIM, :], Act.Tanh,
                    bias=hb2_sb[:, 0:1], scale=1.0 / P_SC,
                )
                d = small.tile([ACTION_DIM, NT], f32, tag="d")
                nc.vector.tensor_sub(d[:], mean[:], aT_sb[:, ts])
                sq = small.tile([ACTION_DIM, NT], f32r, tag="sq")
                nc.vector.tensor_mul(sq[:], d[:], d[:])
                pl = psmall.tile([1, NT], f32, tag="pl")
                nc.tensor.matmul(pl[:], wred_sb[:], sq[:], start=True, stop=True)
                o = small.tile([1, NT], f32, tag="o")
                nc.vector.tensor_scalar_add(o[:], pl[:], cc_sb[0:1, 0:1])
                nc.sync.dma_start(out[t : t + 1, :], o[:])

            def emit_l0(dst, t_idx):
                for m in range(KH):
                    ps = psum.tile([128, NT], f32, tag="ps")
                    nc.tensor.matmul(ps[:], w0_sb[:, bass.ts(m, 128)],
                                     xt_sb[t_idx][:], start=True, stop=True)
                    emit_relu(dst, m, ps, b0_sb, False, MAP_L0)

            def emit_dr_group(w_sb, src, ps, m, nkk):
                wm = w_sb[:, :, :, bass.ts(m, 128)]
                for kk in range(nkk):
                    nc.tensor.matmul(
                        ps[:], wm[:, kk, :, :], src[:, kk, :, :],
                        start=(kk == 0), stop=(kk == nkk - 1), perf_mode=DR,
                    )

            def emit_h2(src):
                pm = psmall.tile([MPAD, NT], f32, tag="pm")
                for kk in range(KKZ):
                    nc.tensor.matmul(
                        pm[:], hw2_sb[:, kk, :, :], src[:, kk, :, :],
                        start=(kk == 0), stop=(kk == KKZ - 1), perf_mode=DR,
                    )
                return pm

            loop_cm = tc.For_i(0, repeats, 1) if repeats > 1 else nullcontext()
            with loop_cm:
                pending = []
                if PAIRED:
                    for p in range(CT // 2):
                        tA, tB = 2 * p, 2 * p + 1

                        h0A = acts.tile([128, KKH, 2, NT], fp8, tag="h0")
                        h0B = acts.tile([128, KKH, 2, NT], fp8, tag="h0")
                        for m in range(KH):
                            wblk = w0_sb[:, bass.ts(m, 128)]
                            psA = psum.tile([128, NT], f32, tag="ps")
                            nc.tensor.matmul(psA[:], wblk, xt_sb[tA][:],
                                             start=True, stop=True)
                            emit_relu(h0A, m, psA, b0_sb, False, MAP_L0)
                            psB = psum.tile([128, NT], f32, tag="ps")
                            nc.tensor.matmul(psB[:], wblk, xt_sb[tB][:],
                                             start=True, stop=True)
                            emit_relu(h0B, m, psB, b0_sb, False, MAP_L0)

                        def dr_layer(w_sb, src_A, src_B, dst_A, dst_B,
                                     bias_sb, emap, nblk, nkk):
                            for m in range(nblk):
                                psA = psum.tile([128, NT], f32, tag="ps")
                                emit_dr_group(w_sb, src_A, psA, m, nkk)
                                emit_relu(dst_A, m, psA, bias_sb, True, emap)
                                psB = psum.tile([128, NT], f32, tag="ps")
                                emit_dr_group(w_sb, src_B, psB, m, nkk)
                                emit_relu(dst_B, m, psB, bias_sb, True, emap)

                        h1A = acts.tile([128, KKH, 2, NT], fp8, tag="h1")
                        h1B = acts.tile([128, KKH, 2, NT], fp8, tag="h1")
                        dr_layer(w1_sb, h0A, h0B, h1A, h1B, b1_sb, MAP_L1, KH, KKH)

                        h2A = acts.tile([128, KKH, 2, NT], fp8, tag="h2")
                        h2B = acts.tile([128, KKH, 2, NT], fp8, tag="h2")
                        dr_layer(w2_sb, h1A, h1B, h2A, h2B, b2_sb, MAP_L2, KH, KKH)

                        # deferred tails from the previous pair: flushed here so
                        # the tanh/diff/square chain has two full layers of PE
                        # work as slack before the lp matmul needs it
                        while pending:
                            emit_tail(*pending.pop(0))

                        zA = acts.tile([128, KKZ, 2, NT], fp8, tag="z")
                        zB = acts.tile([128, KKZ, 2, NT], fp8, tag="z")
                        dr_layer(hw1_sb, h2A, h2B, zA, zB, hb1_sb, MAP_H1, KZ, KKH)

                        pending.append((tA, emit_h2(zA)))
                        pending.append((tB, emit_h2(zB)))
                        if not DEFER_TAILS:
                            while pending:
                                emit_tail(*pending.pop(0))
                else:
                    for t in range(CT):
                        h0 = acts.tile([128, KKH, 2, NT], fp8, tag="h0")
                        emit_l0(h0, t)
                        while pending:
                            emit_tail(*pending.pop(0))
                        h1 = acts.tile([128, KKH, 2, NT], fp8, tag="h1")
                        for m in range(KH):
                            ps = psum.tile([128, NT], f32, tag="ps")
                            emit_dr_group(w1_sb, h0, ps, m, KKH)
                            emit_relu(h1, m, ps, b1_sb, True, MAP_L1)
                        h2 = acts.tile([128, KKH, 2, NT], fp8, tag="h2")
                        for m in range(KH):
                            ps = psum.tile([128, NT], f32, tag="ps")
                            emit_dr_group(w2_sb, h1, ps, m, KKH)
                            emit_relu(h2, m, ps, b2_sb, True, MAP_L2)
                        z = acts.tile([128, KKZ, 2, NT], fp8, tag="z")
                        for m in range(KZ):
                            ps = psum.tile([128, NT], f32, tag="ps")
                            emit_dr_group(hw1_sb, h2, ps, m, KKH)
                            emit_relu(z, m, ps, hb1_sb, True, MAP_H1)
                        pending.append((t, emit_h2(z)))
                while pending:
                    emit_tail(*pending.pop(0))

    nc.compile()
    return nc


def _get_graph(repeats=1):
    if repeats not in _GRAPHS:
        _GRAPHS[repeats] = _build_graph(repeats)
    return _GRAPHS[repeats]


def _round_f32r(a):
    """Round fp32 to the PE's fp32r format (11-bit mantissa, low 12 bits 0)."""
    b = np.ascontiguousarray(a, dtype=np.float32).view(np.uint32)
    lsb = (b >> np.uint32(12)) & np.uint32(1)
    out = (b + np.uint32(0x7FF) + lsb) & np.uint32(0xFFFFF000)
    return out.view(np.float32)


def _fp8(a):
    return np.ascontiguousarray(np.asarray(a, dtype=np.float32).astype(F8))


def _bf16(a):
    return np.ascontiguousarray(np.asarray(a, dtype=np.float32).astype(BF16))


def _block_scales(emap, layer0=False):
    """Stored-activation scale per output block for a layer."""
    if layer0:
        return np.array([ALPHA if a else 1.0 for a in emap], np.float32)
    return np.array([ALPHA if a else P_SC for a in emap], np.float32)


def _quant_w(W, s_in):
    """W: [K, M] -> fp8(c_k * W), c_k = P / s_in[block(k)], in DoubleRow
    layout [128, K//256, 2, M]."""
    K, M = W.shape
    c = np.repeat(P_SC / s_in, 128)[:, None]
    Wq = _fp8(c * W)
    return np.ascontiguousarray(
        Wq.reshape(K // 256, 2, 128, M).transpose(2, 0, 1, 3)
    )


def _bias_cols(b, emap, layer0=False):
    """[128, nblk] f32, column m pre-scaled for its engine path."""
    nblk = len(emap)
    cols = b.reshape(nblk, 128).T.astype(np.float32).copy()
    for m, is_act in enumerate(emap):
        cols[:, m] *= ALPHA if is_act else (1.0 if layer0 else P_SC)
    return np.ascontiguousarray(cols)


def _pack(obs_f, act_f, tok_pad):
    """Gather+transpose per-core token data: xT as [CT, 128, NT] contiguous
    per-tile blocks (bf16), aT as [8, C] f32."""
    xT = np.ascontiguousarray(
        obs_f[tok_pad].T.reshape(128, CT, NT).transpose(1, 0, 2)
    )
    aT = np.ascontiguousarray(act_f[tok_pad].T)
    return xT, aT


def kernel(
    obs, role_ids, actions,
    W0, b0, W1, b1, W2, b2,
    hW1, hb1, hW2, hb2, log_stds,
):
    from concourse.bass_utils import run_bass_kernel_spmd

    obs = np.asarray(obs, dtype=np.float32)
    role_ids = np.asarray(role_ids)
    actions = np.asarray(actions, dtype=np.float32)
    W0 = np.asarray(W0, dtype=np.float32)
    b0 = np.asarray(b0, dtype=np.float32)
    W1 = np.asarray(W1, dtype=np.float32)
    b1 = np.asarray(b1, dtype=np.float32)
    W2 = np.asarray(W2, dtype=np.float32)
    b2 = np.asarray(b2, dtype=np.float32)
    hW1 = np.asarray(hW1, dtype=np.float32)
    hb1 = np.asarray(hb1, dtype=np.float32)
    hW2 = np.asarray(hW2, dtype=np.float32)
    hb2 = np.asarray(hb2, dtype=np.float32)
    log_stds = np.asarray(log_stds, dtype=np.float32)

    nb, na = role_ids.shape
    obs_f = _round_f32r(obs.reshape(-1, OBS_DIM))
    act_f = actions.reshape(-1, ACTION_DIM)
    roles_f = role_ids.reshape(-1)
    n_tok = roles_f.shape[0]

    # ---- tile lists per role (token index + scatter destination) ----------
    tiles = []
    for r in range(NUM_ROLES):
        idx = np.nonzero(roles_f == r)[0]
        n = idx.shape[0]
        for s in range(0, n, NT):
            chunk = idx[s : s + NT]
            tok = np.zeros(NT, dtype=np.int64)
            dst = np.full(NT, -1, dtype=np.int64)
            tok[: chunk.shape[0]] = chunk
            dst[: chunk.shape[0]] = chunk
            tiles.append((r, tok, dst))

    out_full = np.zeros(n_tok, dtype=np.float32)
    if not tiles:
        return out_full.reshape(nb, na)

    # ---- static weight payloads -------------------------------------------
    s0 = _block_scales(MAP_L0, layer0=True)
    s1 = _block_scales(MAP_L1)
    s2 = _block_scales(MAP_L2)
    s3 = _block_scales(MAP_H1)

    w0_dev = _round_f32r(W0[:OBS_DIM])
    w1_dev = _quant_w(W1, s0)
    w2_dev = _quant_w(W2, s1)
    b1_dev = _bias_cols(b1, MAP_L1)
    b2_dev = _bias_cols(b2, MAP_L2)

    role_payload = {}
    for r in range(NUM_ROLES):
        b0p = b0 + W0[OBS_DIM + r]
        inv_std = np.exp(-log_stds[r]).astype(np.float64)
        wred_v = (-0.5 * inv_std * inv_std).astype(np.float32)
        c_v = np.float32(-np.sum(log_stds[r]) - 0.5 * LOG_2PI * ACTION_DIM)
        hw2_pad = np.zeros((H2, MPAD), np.float32)
        hw2_pad[:, :ACTION_DIM] = hW2[r]
        role_payload[r] = dict(
            w0=w0_dev,
            bias0=_bias_cols(b0p, MAP_L0, layer0=True),
            w1=w1_dev, bias1=b1_dev,
            w2=w2_dev, bias2=b2_dev,
            hw1=_quant_w(hW1[r], s2),
            hbias1=_bias_cols(hb1[r], MAP_H1),
            hw2=_quant_w(hw2_pad, s3),
            hb2=np.ascontiguousarray(hb2[r].reshape(ACTION_DIM, 1)),
            wred=wred_v.reshape(ACTION_DIM, 1),
            cc=np.full((1, 1), c_v, dtype=np.float32),
        )

    nc = _get_graph(int(getattr(kernel, "nrep", 1)))

    # ---- pack tiles into batches of N_CORES cores x CT single-role tiles --
    batches = []
    i = 0
    while i < len(tiles):
        cores = []
        for _ in range(N_CORES):
            if i >= len(tiles):
                cores.append((0, []))
                continue
            role = tiles[i][0]
            group = []
            while i < len(tiles) and tiles[i][0] == role and len(group) < CT:
                group.append(tiles[i][1:])
                i += 1
            cores.append((role, group))
        batches.append(cores)

    for cores in batches:
        in_maps = []
        scatter = []
        for ci, (role, group) in enumerate(cores):
            toks = [g[0] for g in group]
            dsts = [g[1] for g in group]
            while len(toks) < CT:
                toks.append(np.zeros(NT, dtype=np.int64))
                dsts.append(np.full(NT, -1, dtype=np.int64))
            tok_pad = np.concatenate(toks)
            dst_pad = np.concatenate(dsts)
            xT_c, aT_c = _pack(obs_f, act_f, tok_pad)
            m = dict(role_payload[role])
            m["xT"] = xT_c
            m["aT"] = aT_c
            in_maps.append(m)
            scatter.append(dst_pad)

        res = run_bass_kernel_spmd(nc, in_maps, list(range(N_CORES)))
        for ci in range(N_CORES):
            vals = np.asarray(res.results[ci]["out"]).reshape(-1)
            dst = scatter[ci]
            valid = dst >= 0
            out_full[dst[valid]] = vals[valid]

    return out_full.reshape(nb, na)

